# revision 19
# baseline (speedup 1.0000x reference)
"""LSTM-cell scan (masked Encoder) on 8 Trainium2 NeuronCores.

Problem: sequence [B=256, A=128, T=50, I=16] f32 through a single
LSTMCell (H=128) scanned over T; returns final (h, c), each [B, A, H].
The mask input is all-ones (per the problem spec), so the mask blend is
an identity and is skipped on-device (a host-side check falls back to a
numpy implementation if a non-trivial mask ever shows up).

Sharding: data-parallel over B across 8 cores (32 batch rows -> 4096
independent sequences per core). Params replicated. The shard layout is
time-major x^T ([T, 16+1, N] with a constant ones-row per step), so the
device streams it straight into SBUF.

Per-core layout: H=128 on SBUF partitions, 4096 sequences on the free
dim. Per timestep the batch is processed in 8 sub-tiles of 512 (PSUM
bank width); all four gate pre-activations of a sub-tile live in ONE
4-bank PSUM tile, computed as accumulating fp32r matmuls:
    psum[:, g*512:(g+1)*512] = Wx_g^T.T @ [x_t; 1] (+) Wh_g^T.T @ h
The bias rides the x-part matmul as a 17th contraction row against the
ones-row. The g-gate weights are doubled host-side so
tanh(z) = 2*sigmoid(2z)-1 turns ALL gates into sigmoids, letting a
single [128, 2048] ACT op activate the whole sub-tile. The affine
corrections fold into scalar_tensor_tensor ops:
    t1 = (sg2 - 0.5) * si ;  c = 2*t1 + sf*c ;  h = so * tanh(c)
with the f*c product and half the h products on the Pool engine.
States h (fp32r) and c (fp32) persist in SBUF across all 50 steps.
x^T rows per timestep sit at partition bases {0,32,64} (the only legal
matmul bases), three timesteps per staged chunk.
"""

import os
from contextlib import ExitStack

import numpy as np

N_CORES = 8
B, A, T, I, H = 256, 128, 50, 16, 128
NB = B // N_CORES          # batch rows per core
N = NB * A                 # sequences per core (4096)
NSUB = 8                   # matmul sub-tiles per core
NS = N // NSUB             # sub-tile width (512, PSUM bank)
NP = 4                     # state pair-tiles per core
PW = N // NP               # pair width (1024)
TC = 3                     # timesteps per staged x chunk
NCH = (T + TC - 1) // TC   # chunks (17: sixteen of 3, one of 2)
KX = I + 1                 # x-part contraction (16 x rows + ones row)


def _build_nc():
    import concourse.bacc as bacc
    import concourse.tile as tile
    from concourse import mybir

    F32 = mybir.dt.float32
    F32R = mybir.dt.float32r
    ACTF = mybir.ActivationFunctionType
    ALU = mybir.AluOpType

    nc = bacc.Bacc("TRN2", target_bir_lowering=False, debug=True)

    seqt = nc.dram_tensor("seqt", [T, KX, N], F32R, kind="ExternalInput")
    wih_p = nc.dram_tensor("wih_p", [TC * 32, 4 * H], F32R, kind="ExternalInput")
    whh_t = nc.dram_tensor("whh_t", [H, 4 * H], F32R, kind="ExternalInput")
    h_out = nc.dram_tensor("h_out", [H, N], F32, kind="ExternalOutput")
    c_out = nc.dram_tensor("c_out", [H, N], F32, kind="ExternalOutput")

    with tile.TileContext(nc) as tc, ExitStack() as ctx:
        consts = ctx.enter_context(tc.tile_pool(name="consts", bufs=1))
        state = ctx.enter_context(tc.tile_pool(name="state", bufs=1))
        xchunk = ctx.enter_context(tc.tile_pool(name="xchunk", bufs=2))
        gates = ctx.enter_context(tc.tile_pool(name="gates", bufs=6))
        tmps = ctx.enter_context(tc.tile_pool(name="tmps", bufs=4))
        psum = ctx.enter_context(tc.tile_pool(name="psum", bufs=2, space="PSUM"))

        # --- weights (fp32r straight from DRAM) --------------------------
        wih = consts.tile([TC * 32, 4 * H], F32R)
        whh = consts.tile([H, 4 * H], F32R)
        nc.sync.dma_start(out=wih, in_=wih_p[:, :])
        nc.sync.dma_start(out=whh, in_=whh_t[:, :])

        # --- persistent state -------------------------------------------
        h_st = [state.tile([H, PW], F32R, tag=f"h{k}", name=f"h{k}")
                for k in range(NP)]
        c_st = [state.tile([H, PW], F32, tag=f"c{k}", name=f"c{k}")
                for k in range(NP)]

        pend = []
        subt = 0

        def emit_tanh_h(item):
            _, t, p, so_e, so_o = item
            tc_t = tmps.tile([H, PW], F32, tag="tc", name=f"tc_{t}_{p}")
            nc.scalar.activation(out=tc_t, in_=c_st[p], func=ACTF.Tanh)
            if p % 2 == 0:
                nc.gpsimd.tensor_mul(
                    out=h_st[p][:, 0:NS], in0=so_e, in1=tc_t[:, 0:NS]
                )
                nc.vector.tensor_mul(
                    out=h_st[p][:, NS:PW], in0=so_o, in1=tc_t[:, NS:PW]
                )
            else:
                nc.vector.tensor_mul(
                    out=h_st[p][:, 0:NS], in0=so_e, in1=tc_t[:, 0:NS]
                )
                nc.gpsimd.tensor_mul(
                    out=h_st[p][:, NS:PW], in0=so_o, in1=tc_t[:, NS:PW]
                )

        for j in range(NCH):
            tsteps = min(TC, T - j * TC)
            # stage this chunk of x^T: timestep ts lands on partitions
            # [32ts, 32ts+17) (x rows + ones row) per pair tile
            xT = [
                xchunk.tile([TC * 32, PW], F32R, tag=f"xT{p}", name=f"xT{j}_{p}")
                for p in range(NP)
            ]
            for ts in range(tsteps):
                t = j * TC + ts
                for p in range(NP):
                    nc.sync.dma_start(
                        out=xT[p][ts * 32:ts * 32 + KX, :],
                        in_=seqt[t, :, p * PW:(p + 1) * PW],
                    )

            for ts in range(tsteps):
                t = j * TC + ts
                wsl = wih[ts * 32:ts * 32 + KX, :]
                for k2 in range(NSUB):
                    p = k2 // 2        # pair index
                    lo = (k2 % 2) * NS  # offset within pair tiles
                    ps = psum.tile([H, 4 * NS], F32, tag="ps",
                                   name=f"ps_{t}_{k2}")
                    # h-part first: it opens the accumulation group, so the
                    # PSUM slot is only claimed once h is ready (short hold)
                    if t > 0:
                        for g in range(4):
                            nc.tensor.matmul(
                                ps[:, g * NS:(g + 1) * NS],
                                whh[:, g * H:(g + 1) * H],
                                h_st[p][:, lo:lo + NS],
                                start=True,
                                stop=False,
                            )
                    for g in range(4):
                        nc.tensor.matmul(
                            ps[:, g * NS:(g + 1) * NS],
                            wsl[:, g * H:(g + 1) * H],
                            xT[p][ts * 32:ts * 32 + KX, lo:lo + NS],
                            start=(t == 0),
                            stop=True,
                        )
                    G = gates.tile([H, 4 * NS], F32, tag="G", name=f"G_{t}_{k2}")
                    nc.scalar.activation(out=G, in_=ps, func=ACTF.Sigmoid)
                    si = G[:, 0:NS]
                    sf = G[:, NS:2 * NS]
                    sg2 = G[:, 2 * NS:3 * NS]
                    so = G[:, 3 * NS:4 * NS]
                    c_sl = c_st[p][:, lo:lo + NS]
                    # t1 = (sigma(2g) - 0.5) * sigma(i)  [= tanh(g)*si / 2]
                    t1 = tmps.tile([H, NS], F32, tag="t1", name=f"t1_{t}_{k2}")
                    nc.vector.scalar_tensor_tensor(
                        out=t1, in0=sg2, scalar=-0.5, in1=si,
                        op0=ALU.add, op1=ALU.mult,
                    )
                    if t == 0:
                        # c0 = 2*t1
                        nc.vector.tensor_scalar_mul(c_sl, t1, 2.0)
                    else:
                        c2 = tmps.tile([H, NS], F32, tag="c2",
                                       name=f"c2_{t}_{k2}")
                        nc.gpsimd.tensor_mul(out=c2, in0=sf, in1=c_sl)
                        # c = 2*t1 + c2
                        nc.vector.scalar_tensor_tensor(
                            out=c_sl, in0=t1, scalar=2.0, in1=c2,
                            op0=ALU.mult, op1=ALU.add,
                        )
                    if k2 % 2 == 0:
                        prev_so = so
                    else:
                        pend.append([subt, t, p, prev_so, so])
                    subt += 1
                    # emit tanh+h for pairs whose c completed >=2 slots ago;
                    # keeps ACT's in-order queue free of ops that would
                    # head-of-line block behind the DVE cell-update chain
                    while pend and subt - pend[0][0] >= 4:
                        emit_tanh_h(pend.pop(0))

        while pend:
            emit_tanh_h(pend.pop(0))

        for k in range(NP):
            nc.sync.dma_start(
                out=h_out[:, k * PW:(k + 1) * PW], in_=h_st[k].bitcast(F32)
            )
            nc.sync.dma_start(
                out=c_out[:, k * PW:(k + 1) * PW], in_=c_st[k]
            )

    nc.finalize()
    return nc


def _numpy_fallback(sequence, mask, W_ih, W_hh, b_ih, b_hh):
    nb, na, nt, _ = sequence.shape
    hdim = W_hh.shape[1]
    h = np.zeros((nb, na, hdim), np.float32)
    c = np.zeros((nb, na, hdim), np.float32)
    bias = (b_ih + b_hh).astype(np.float32)

    def sig(x):
        return 1.0 / (1.0 + np.exp(-x))

    for t in range(nt):
        x = sequence[:, :, t, :]
        gates = x @ W_ih.T + h @ W_hh.T + bias
        i_g = gates[..., 0 * hdim:1 * hdim]
        f_g = gates[..., 1 * hdim:2 * hdim]
        g_g = gates[..., 2 * hdim:3 * hdim]
        o_g = gates[..., 3 * hdim:4 * hdim]
        c_new = sig(f_g) * c + sig(i_g) * np.tanh(g_g)
        h_new = sig(o_g) * np.tanh(c_new)
        m = mask[:, :, t][..., None]
        h = m * h_new + (1.0 - m) * h
        c = m * c_new + (1.0 - m) * c
    return h, c


def kernel(sequence, mask, W_ih, W_hh, b_ih, b_hh):
    sequence = np.asarray(sequence, dtype=np.float32)
    mask = np.asarray(mask, dtype=np.float32)
    W_ih = np.asarray(W_ih, dtype=np.float32)
    W_hh = np.asarray(W_hh, dtype=np.float32)
    b_ih = np.asarray(b_ih, dtype=np.float32)
    b_hh = np.asarray(b_hh, dtype=np.float32)

    if not np.all(mask == 1.0):
        return _numpy_fallback(sequence, mask, W_ih, W_hh, b_ih, b_hh)

    from concourse.bass_utils import run_bass_kernel_spmd

    # double the g-gate block so tanh(z) = 2*sigmoid(2z) - 1
    gscale = np.ones((4 * H, 1), np.float32)
    gscale[2 * H:3 * H] = 2.0
    bias = ((b_ih + b_hh).reshape(4 * H, 1) * gscale).reshape(4 * H)
    wih_sc = W_ih * gscale                     # [4H, I]
    whh_sc = W_hh * gscale                     # [4H, H]
    # packed x-part lhsT at partition bases 0/32/64: 16 W rows + bias row
    wih_pk = np.zeros((TC * 32, 4 * H), np.float32)
    for r in range(TC):
        wih_pk[r * 32:r * 32 + I] = wih_sc.T
        wih_pk[r * 32 + I] = bias
    whh_tp = np.ascontiguousarray(whh_sc.T)    # [H, 4H]

    # time-major x^T shards with the ones (bias) row appended per step
    seq_bat = sequence.reshape(B * A, T, I)
    in_maps = []
    for cidx in range(N_CORES):
        xc = seq_bat[cidx * N:(cidx + 1) * N]            # [N, T, I]
        seqt = np.empty((T, KX, N), np.float32)
        seqt[:, :I, :] = xc.transpose(1, 2, 0)
        seqt[:, I, :] = 1.0
        in_maps.append({
            "seqt": seqt,
            "wih_p": wih_pk,
            "whh_t": whh_tp,
        })

    nc = _build_nc()
    kernel.last_nc = nc
    trace = bool(int(os.environ.get("LSTM_KERNEL_TRACE", "0")))
    res = run_bass_kernel_spmd(
        nc, in_maps, core_ids=list(range(N_CORES)), trace=trace
    )
    if trace and res.exec_time_ns is not None:
        print(f"HW exec time: {res.exec_time_ns} ns")
        kernel.last_exec_time_ns = res.exec_time_ns
        kernel.last_trace = res.instructions_and_trace
    h_full = np.empty((B, A, H), np.float32)
    c_full = np.empty((B, A, H), np.float32)
    for cidx in range(N_CORES):
        hT = res.results[cidx]["h_out"]  # [H, N]
        cT = res.results[cidx]["c_out"]
        h_full[cidx * NB:(cidx + 1) * NB] = hT.T.reshape(NB, A, H)
        c_full[cidx * NB:(cidx + 1) * NB] = cT.T.reshape(NB, A, H)
    return h_full, c_full
